# revision 15
# baseline (speedup 1.0000x reference)
import numpy as np

import concourse.bass as bass
import concourse.tile as tile
from concourse import bacc, mybir
from concourse.bass_utils import run_bass_kernel_spmd
from concourse.masks import make_identity

F32 = mybir.dt.float32
BF16 = mybir.dt.bfloat16
AL = mybir.AluOpType
AF = mybir.ActivationFunctionType

P = 128
N_FULL = 16384
N_HALF = 8192
C = 256
HD = 512
OUT = 256
EPS = 1e-5
CH_ROWS = 1024
N_CHUNKS = N_HALF // CH_ROWS
SUBT = CH_ROWS // P
G_TILES = N_HALF // P
INV_N = 1.0 / float(N_FULL)
REPLICA_GROUPS = [[0, 1], [2, 3], [4, 5], [6, 7]]


def build_nc():
    nc = bacc.Bacc(
        "TRN2",
        target_bir_lowering=False,
        debug=False,
        num_devices=8,
    )
    u_d = nc.dram_tensor("u", [N_HALF, C], F32, kind="ExternalInput").ap()
    wq_d = nc.dram_tensor("wq", [HD, C], F32, kind="ExternalInput").ap()
    wk_d = nc.dram_tensor("wk", [HD, C], F32, kind="ExternalInput").ap()
    wv_d = nc.dram_tensor("wv", [HD, C], F32, kind="ExternalInput").ap()
    wo_d = nc.dram_tensor("wo", [OUT, HD], F32, kind="ExternalInput").ap()
    out_d = nc.dram_tensor("out", [N_HALF, OUT], F32, kind="ExternalOutput").ap()
    cc_in = nc.dram_tensor("cc_in", [P, 2 * (C + 1)], F32)
    cc_out = nc.dram_tensor("cc_out", [P, 2 * (C + 1)], F32)

    with tile.TileContext(nc) as tc:
        with tc.tile_pool(name="pers", bufs=1) as pers:
            ubf = pers.tile([P, G_TILES, C + 1], BF16)
            uT = pers.tile([P, 2, N_HALF], BF16)
            ident = pers.tile([P, P], F32)
            make_identity(nc, ident[:])
            ident_bf = pers.tile([P, P], BF16)
            nc.vector.tensor_copy(ident_bf[:], ident[:])
            wq_bf = pers.tile([P, 4, C], BF16)
            wkT_bf = pers.tile([P, 2, HD], BF16)
            wvT_bf = pers.tile([P, 2, HD], BF16)
            woT_bf = pers.tile([P, 4, OUT], BF16)
            weff = pers.tile([P, 2, OUT], BF16)
            cuu2 = pers.tile([P, 2, C + 1], F32)
            cuu_bf = pers.tile([P, 2, C + 1], BF16)
            ones_col_f = pers.tile([P, 1], F32)
            nc.vector.memset(ones_col_f[:], 1.0)
            ones_row = pers.tile([1, P], F32)
            nc.vector.memset(ones_row[:], 1.0)
            one1 = pers.tile([1, 1], F32)
            nc.vector.memset(one1[:], 1.0)
            warm = pers.tile([1, 8], F32)
            nc.vector.memset(warm[:], 1.0)
            nc.scalar.mul(warm[:], warm[:], 1.0)
            nc.scalar.activation(warm[:], warm[:], AF.Sqrt)

            with (
                tc.tile_pool(name="upool", bufs=3) as upool,
                tc.tile_pool(name="pacc", bufs=1, space="PSUM") as pacc,
                tc.tile_pool(name="wstage", bufs=2) as wst,
                tc.tile_pool(name="wps", bufs=2, space="PSUM") as wps,
                tc.tile_pool(name="ptr", bufs=3, space="PSUM") as ptr,
            ):
                cps = [
                    pacc.tile([P, C + 1], F32, tag=f"c{t}", name=f"c{t}")
                    for t in range(2)
                ]
                for ch in range(N_CHUNKS):
                    u1 = upool.tile([P, SUBT, C + 1], F32, tag="u1", name="u1")
                    nc.sync.dma_start(
                        u1[:, :, 0:C],
                        u_d[ch * CH_ROWS:(ch + 1) * CH_ROWS, :].rearrange(
                            "(p j) c -> p j c", p=P
                        ),
                    )
                    nc.vector.memset(u1[:, :, C:C + 1], 1.0)
                    dst = ubf[:, ch * SUBT:(ch + 1) * SUBT, :]
                    nc.vector.tensor_copy(dst, u1[:])
                    for j in range(SUBT):
                        g = ch * SUBT + j
                        for t in range(2):
                            nc.tensor.matmul(
                                cps[t][:],
                                ubf[:, g, t * P:(t + 1) * P],
                                ubf[:, g, :],
                                start=(g == 0),
                                stop=(g == G_TILES - 1),
                            )

                cuu = pers.tile([P, 2, C + 1], F32)
                for t in range(2):
                    nc.any.tensor_copy(cuu[:, t, :], cps[t][:])

                nc.gpsimd.dma_start(
                    cc_in.ap(), cuu[:].rearrange("p t c -> p (t c)")
                )
                nc.gpsimd.collective_compute(
                    "AllReduce",
                    AL.add,
                    ins=[cc_in.ap().opt()],
                    outs=[cc_out.ap().opt()],
                    replica_groups=REPLICA_GROUPS,
                )
                nc.sync.dma_start(
                    cuu2[:].rearrange("p t c -> p (t c)"), cc_out.ap()
                )
                nc.vector.tensor_copy(cuu_bf[:], cuu2[:])

                nc.sync.dma_start(
                    wq_bf0 := wst.tile([P, 4, C], F32, tag="wnat", name="wq_f"),
                    wq_d.rearrange("(a p) c -> p a c", p=P),
                )
                nc.vector.tensor_copy(wq_bf[:], wq_bf0[:])
                for w_d, wT_t in ((wk_d, wkT_bf), (wv_d, wvT_bf)):
                    wnat = wst.tile([P, 4, C], F32, tag="wnat", name="wnat")
                    nc.sync.dma_start(
                        wnat[:], w_d.rearrange("(a p) c -> p a c", p=P)
                    )
                    for a in range(4):
                        for b2 in range(2):
                            pst = wps.tile([P, P], F32, tag="wt", name="pst")
                            nc.tensor.transpose(
                                pst[:], wnat[:, a, b2 * P:(b2 + 1) * P], ident[:]
                            )
                            nc.any.tensor_copy(
                                wT_t[:, b2, a * P:(a + 1) * P], pst[:]
                            )
                wonat = wst.tile([P, 2, HD], F32, tag="wonat", name="wonat")
                nc.sync.dma_start(
                    wonat[:], wo_d.rearrange("(a p) c -> p a c", p=P)
                )
                for a in range(2):
                    for b2 in range(4):
                        pst = wps.tile([P, P], F32, tag="wt", name="pst")
                        nc.tensor.transpose(
                            pst[:], wonat[:, a, b2 * P:(b2 + 1) * P], ident[:]
                        )
                        nc.any.tensor_copy(
                            woT_bf[:, b2, a * P:(a + 1) * P], pst[:]
                        )
                for g in range(G_TILES):
                    tps = ptr.tile([P, 2 * P], BF16, tag="uT", name="tps")
                    for t in range(2):
                        nc.tensor.transpose(
                            tps[:, t * P:(t + 1) * P],
                            ubf[:, g, t * P:(t + 1) * P],
                            ident_bf[:],
                        )
                    nc.vector.tensor_copy(
                        uT[:, :, g * P:(g + 1) * P],
                        tps[:].rearrange("p (t n) -> p t n", t=2),
                    )

            with tc.tile_pool(name="sm", bufs=1) as sm:
                psA_ctx = tc.tile_pool(name="psA", bufs=1, space="PSUM")
                psA = psA_ctx.__enter__()
                a_k = sm.tile([P, 2, HD], BF16)
                m_k = sm.tile([P, 2, HD], F32)
                a_v = sm.tile([P, 2, HD], BF16)
                m_v = sm.tile([P, 2, HD], F32)
                for wT_t, a_t, mm_t in ((wkT_bf, a_k, m_k), (wvT_bf, a_v, m_v)):
                    for t in range(2):
                        aps = psA.tile([P, HD], F32, tag="aps", bufs=2, name="aps")
                        for tp in range(2):
                            nc.tensor.matmul(
                                aps[:],
                                cuu_bf[:, tp, t * P:(t + 1) * P],
                                wT_t[:, tp, :],
                                start=(tp == 0),
                                stop=(tp == 1),
                            )
                        nc.vector.tensor_copy(a_t[:, t, :], aps[:])
                        nc.vector.tensor_mul(mm_t[:, t, :], aps[:], wT_t[:, t, :])

                mk = sm.tile([1, HD], F32)
                mv = sm.tile([1, HD], F32)
                ekk = sm.tile([1, HD], F32)
                evv = sm.tile([1, HD], F32)
                tk = sm.tile([1, HD], F32)
                tv = sm.tile([1, HD], F32)
                vark = sm.tile([1, HD], F32)
                varv = sm.tile([1, HD], F32)
                for wT_t, m_t in ((wkT_bf, mk), (wvT_bf, mv)):
                    sps = psA.tile([1, HD], F32, tag="st", bufs=2, name="sps")
                    for tp in range(2):
                        nc.tensor.matmul(
                            sps[:],
                            cuu_bf[:, tp, C:C + 1],
                            wT_t[:, tp, :],
                            start=(tp == 0),
                            stop=(tp == 1),
                        )
                    nc.scalar.activation(m_t[:], sps[:], AF.Copy, scale=INV_N)
                for m_src, e_t in ((m_k, ekk), (m_v, evv)):
                    sps = psA.tile([1, HD], F32, tag="st", bufs=2, name="sps")
                    for tp in range(2):
                        nc.tensor.matmul(
                            sps[:],
                            ones_col_f[:],
                            m_src[:, tp, :],
                            start=(tp == 0),
                            stop=(tp == 1),
                        )
                    nc.scalar.activation(e_t[:], sps[:], AF.Copy, scale=INV_N)
                nc.vector.tensor_mul(tk[:], mk[:], mk[:])
                nc.vector.tensor_mul(tv[:], mv[:], mv[:])
                nc.vector.tensor_sub(vark[:], ekk[:], tk[:])
                nc.vector.tensor_sub(varv[:], evv[:], tv[:])

                eps_col = sm.tile([P, 4], F32)
                nc.vector.memset(eps_col[:], EPS)
                rk_col = sm.tile([P, 4], F32)
                rv_col = sm.tile([P, 4], F32)
                for var_row, r_col in ((vark, rk_col), (varv, rv_col)):
                    vc = psA.tile([P, 4], F32, tag="vc", bufs=2, name="vc")
                    for g in range(4):
                        nc.tensor.matmul(
                            vc[:, g:g + 1],
                            var_row[0:1, g * P:(g + 1) * P],
                            one1[:],
                            start=True,
                            stop=True,
                        )
                    nc.vector.tensor_add(r_col[:], vc[:], eps_col[:])
                    nc.scalar.activation(r_col[:], r_col[:], AF.Sqrt)
                    nc.vector.reciprocal(r_col[:], r_col[:])
                rk_row = sm.tile([1, HD], F32)
                rk_bc = sm.tile([P, HD], F32)
                rps = psA.tile([1, HD], F32, tag="st", bufs=2, name="rps")
                for g in range(4):
                    nc.tensor.matmul(
                        rps[0:1, g * P:(g + 1) * P],
                        rk_col[:, g:g + 1],
                        ident[:],
                        start=True,
                        stop=True,
                    )
                nc.scalar.mul(rk_row[:], rps[:], 1.0)
                bps = psA.tile([P, HD], F32, tag="aps", bufs=2, name="bps")
                nc.tensor.matmul(bps[:], ones_row[:], rk_row[:], start=True, stop=True)
                nc.any.tensor_copy(rk_bc[:], bps[:])
                psA_ctx.__exit__(None, None, None)

                with tc.tile_pool(name="psP", bufs=1, space="PSUM") as psP:
                    wps2 = [
                        psP.tile([P, OUT], F32, tag=f"weff{t}", name=f"wps{t}")
                        for t in range(2)
                    ]
                    for jp in range(4):
                        sl = slice(jp * P, (jp + 1) * P)
                        sd = psP.tile([P, P], F32, tag="sd", bufs=2, name="sd")
                        for tp in range(2):
                            nc.tensor.matmul(
                                sd[:],
                                wvT_bf[:, tp, sl],
                                a_k[:, tp, sl],
                                start=(tp == 0),
                                stop=(tp == 1),
                            )
                        outr = psP.tile([P, P], F32, tag="outr", bufs=2, name="outr")
                        nc.tensor.matmul(
                            outr[:], mv[:, sl], mk[:, sl], start=True, stop=True
                        )
                        kvp = sm.tile([P, P], F32, tag=f"kv{jp}", name=f"kv{jp}")
                        nc.vector.memset(kvp[:], 0.0)
                        for g in range(2):
                            gs = slice(g * 64, g * 64 + 64)
                            nc.scalar.mul(kvp[gs, gs], sd[gs, gs], INV_N)
                            nc.vector.tensor_sub(
                                kvp[gs, gs], kvp[gs, gs], outr[gs, gs]
                            )
                        nc.vector.tensor_mul(kvp[:], kvp[:], rk_bc[:, sl])
                        kvp_bf = sm.tile([P, P], BF16, tag=f"kvb{jp}", name=f"kvb{jp}")
                        nc.vector.tensor_scalar_mul(
                            kvp_bf[:], kvp[:], rv_col[:, jp:jp + 1]
                        )
                        bps2 = psP.tile([P, OUT], F32, tag="bps2", bufs=2, name="bps2")
                        nc.tensor.matmul(
                            bps2[:], kvp_bf[:], woT_bf[:, jp, :], start=True, stop=True
                        )
                        bsb = sm.tile([P, OUT], BF16, tag="bsb", name="bsb")
                        nc.any.tensor_copy(bsb[:], bps2[:])
                        for t in range(2):
                            nc.tensor.matmul(
                                wps2[t][:],
                                wq_bf[:, jp, t * P:(t + 1) * P],
                                bsb[:],
                                start=(jp == 0),
                                stop=(jp == 3),
                            )
                    for t in range(2):
                        nc.any.tensor_copy(weff[:, t, :], wps2[t][:])

            with (
                tc.tile_pool(name="opool", bufs=2) as opool,
                tc.tile_pool(name="pout", bufs=4, space="PSUM") as pout,
            ):
                for ch in range(N_CHUNKS):
                    osb = opool.tile([P, SUBT, OUT], F32, tag="osb", name="osb")
                    for j in range(SUBT):
                        g = ch * SUBT + j
                        ops = pout.tile([P, OUT], F32, tag="ops", name="ops")
                        for t in range(2):
                            nc.tensor.matmul(
                                ops[:],
                                uT[:, t, g * P:(g + 1) * P],
                                weff[:, t, :],
                                start=(t == 0),
                                stop=(t == 1),
                            )
                        if j % 2 == 0:
                            nc.vector.tensor_copy(osb[:, j, :], ops[:])
                        else:
                            nc.scalar.mul(osb[:, j, :], ops[:], 1.0)
                    nc.sync.dma_start(
                        out_d[ch * CH_ROWS:(ch + 1) * CH_ROWS, :].rearrange(
                            "(p j) c -> p j c", p=P
                        ),
                        osb[:],
                    )

    nc.compile()
    return nc


_NC_CACHE = None


def _get_nc():
    global _NC_CACHE
    if _NC_CACHE is None:
        _NC_CACHE = build_nc()
    return _NC_CACHE


def make_in_maps(u_src, Wq, Wk, Wv, Wo):
    in_maps = []
    for c in range(8):
        b, half = c // 2, c % 2
        u_half = np.ascontiguousarray(
            u_src[b, half * N_HALF:(half + 1) * N_HALF]
        )
        in_maps.append(
            {
                "u": u_half,
                "wq": np.ascontiguousarray(Wq),
                "wk": np.ascontiguousarray(Wk),
                "wv": np.ascontiguousarray(Wv),
                "wo": np.ascontiguousarray(Wo),
            }
        )
    return in_maps


def assemble_output(results, bo):
    out = np.empty((4, N_FULL, OUT), dtype=np.float32)
    for c in range(8):
        b, half = c // 2, c % 2
        out[b, half * N_HALF:(half + 1) * N_HALF] = results[c]["out"]
    if np.any(bo):
        out += bo.reshape(1, 1, OUT)
    return out


def run(inputs, trace=False, tmpdir=None):
    u_src = np.asarray(inputs["u_src"], dtype=np.float32)
    Wq = np.asarray(inputs["Wq"], dtype=np.float32)
    Wk = np.asarray(inputs["Wk"], dtype=np.float32)
    Wv = np.asarray(inputs["Wv"], dtype=np.float32)
    Wo = np.asarray(inputs["Wo"], dtype=np.float32)
    bo = np.asarray(inputs["bo"], dtype=np.float32)
    nc = _get_nc()
    in_maps = make_in_maps(u_src, Wq, Wk, Wv, Wo)
    res = run_bass_kernel_spmd(
        nc, in_maps, core_ids=list(range(8)), trace=trace, tmpdir=tmpdir
    )
    return assemble_output(res.results, bo), res


def kernel(**inputs):
    out, _ = run(inputs, trace=False)
    return out


# revision 17
# speedup vs baseline: 1.1294x; 1.1294x over previous
import numpy as np

import concourse.bass as bass
import concourse.tile as tile
from concourse import bacc, mybir
from concourse.bass_utils import run_bass_kernel_spmd
from concourse.masks import make_identity

F32 = mybir.dt.float32
BF16 = mybir.dt.bfloat16
AL = mybir.AluOpType
AF = mybir.ActivationFunctionType

P = 128
N_FULL = 16384
N_HALF = 8192
C = 256
HD = 512
OUT = 256
EPS = 1e-5
CH_ROWS = 2048
N_CHUNKS = N_FULL // CH_ROWS
MY_CHUNKS = N_HALF // CH_ROWS
SUBT = CH_ROWS // P
G_ALL = N_FULL // P
OCH_ROWS = 2048
ON_CHUNKS = N_HALF // OCH_ROWS
OSUB = OCH_ROWS // P
INV_N = 1.0 / float(N_FULL)


def build_nc():
    nc = bacc.Bacc(
        "TRN2",
        target_bir_lowering=False,
        debug=False,
        num_devices=8,
    )
    u_d = nc.dram_tensor("u", [N_FULL, C], F32, kind="ExternalInput").ap()
    wq_d = nc.dram_tensor("wq", [HD, C], F32, kind="ExternalInput").ap()
    wk_d = nc.dram_tensor("wk", [HD, C], F32, kind="ExternalInput").ap()
    wv_d = nc.dram_tensor("wv", [HD, C], F32, kind="ExternalInput").ap()
    wo_d = nc.dram_tensor("wo", [OUT, HD], F32, kind="ExternalInput").ap()
    out_d = nc.dram_tensor("out", [N_HALF, OUT], F32, kind="ExternalOutput").ap()

    with tile.TileContext(nc) as tc:
        with tc.tile_pool(name="pers", bufs=1) as pers:
            uT = pers.tile([P, 2, N_HALF], BF16)
            ident = pers.tile([P, P], F32)
            make_identity(nc, ident[:])
            ident_bf = pers.tile([P, P], BF16)
            nc.vector.tensor_copy(ident_bf[:], ident[:])
            wq_bf = pers.tile([P, 4, C], BF16)
            wkT_bf = pers.tile([P, 2, HD], BF16)
            wvT_bf = pers.tile([P, 2, HD], BF16)
            woT_bf = pers.tile([P, 4, OUT], BF16)
            weff = pers.tile([P, 2, OUT], BF16)
            cuu = pers.tile([P, 2, C + 1], F32)
            cuu_bf = pers.tile([P, 2, C + 1], BF16)
            ones_col_f = pers.tile([P, 1], F32)
            nc.vector.memset(ones_col_f[:], 1.0)
            ones_row = pers.tile([1, P], F32)
            nc.vector.memset(ones_row[:], 1.0)
            one1 = pers.tile([1, 1], F32)
            nc.vector.memset(one1[:], 1.0)
            warm = pers.tile([1, 8], F32)
            nc.vector.memset(warm[:], 1.0)
            nc.scalar.mul(warm[:], warm[:], 1.0)
            nc.scalar.activation(warm[:], warm[:], AF.Sqrt)

            with (
                tc.tile_pool(name="upool", bufs=3) as upool,
                tc.tile_pool(name="pacc", bufs=1, space="PSUM") as pacc,
                tc.tile_pool(name="wstage", bufs=2) as wst,
                tc.tile_pool(name="wps", bufs=2, space="PSUM") as wps,
                tc.tile_pool(name="ptr", bufs=3, space="PSUM") as ptr,
            ):
                cps = [
                    pacc.tile([P, C + 1], F32, tag=f"c{t}", name=f"c{t}")
                    for t in range(2)
                ]
                for ch in range(N_CHUNKS):
                    u1 = upool.tile([P, SUBT, C + 1], F32, tag="u1", name="u1")
                    nc.sync.dma_start(
                        u1[:, :, 0:C],
                        u_d[ch * CH_ROWS:(ch + 1) * CH_ROWS, :].rearrange(
                            "(p j) c -> p j c", p=P
                        ),
                    )
                    nc.vector.memset(u1[:, :, C:C + 1], 1.0)
                    ubf = upool.tile([P, SUBT, C + 1], BF16, tag="ubf", name="ubf")
                    nc.vector.tensor_copy(ubf[:], u1[:])
                    for j in range(SUBT):
                        g = ch * SUBT + j
                        for t in range(2):
                            nc.tensor.matmul(
                                cps[t][:],
                                ubf[:, j, t * P:(t + 1) * P],
                                ubf[:, j, :],
                                start=(g == 0),
                                stop=(g == G_ALL - 1),
                            )
                        if ch < MY_CHUNKS:
                            tps = ptr.tile([P, 2 * P], BF16, tag="uT", name="tps")
                            for t in range(2):
                                nc.tensor.transpose(
                                    tps[:, t * P:(t + 1) * P],
                                    ubf[:, j, t * P:(t + 1) * P],
                                    ident_bf[:],
                                )
                            nc.vector.tensor_copy(
                                uT[:, :, g * P:(g + 1) * P],
                                tps[:].rearrange("p (t n) -> p t n", t=2),
                            )

                wq_f = wst.tile([P, 4, C], F32, tag="wnat", name="wq_f")
                nc.sync.dma_start(wq_f[:], wq_d.rearrange("(a p) c -> p a c", p=P))
                nc.vector.tensor_copy(wq_bf[:], wq_f[:])
                for w_d, wT_t in ((wk_d, wkT_bf), (wv_d, wvT_bf)):
                    wnat = wst.tile([P, 4, C], F32, tag="wnat", name="wnat")
                    nc.sync.dma_start(
                        wnat[:], w_d.rearrange("(a p) c -> p a c", p=P)
                    )
                    wnat_bf = wst.tile([P, 4, C], BF16, tag="wnbf", name="wnat_bf")
                    nc.vector.tensor_copy(wnat_bf[:], wnat[:])
                    for a in range(4):
                        for b2 in range(2):
                            pst = wps.tile([P, P], BF16, tag="wt", name="pst")
                            nc.tensor.transpose(
                                pst[:], wnat_bf[:, a, b2 * P:(b2 + 1) * P],
                                ident_bf[:],
                            )
                            nc.any.tensor_copy(
                                wT_t[:, b2, a * P:(a + 1) * P], pst[:]
                            )
                wonat = wst.tile([P, 2, HD], F32, tag="wonat", name="wonat")
                nc.sync.dma_start(
                    wonat[:], wo_d.rearrange("(a p) c -> p a c", p=P)
                )
                wonat_bf = wst.tile([P, 2, HD], BF16, tag="wnbf", name="wonat_bf")
                nc.vector.tensor_copy(wonat_bf[:], wonat[:])
                for a in range(2):
                    for b2 in range(4):
                        pst = wps.tile([P, P], BF16, tag="wt", name="pst")
                        nc.tensor.transpose(
                            pst[:], wonat_bf[:, a, b2 * P:(b2 + 1) * P],
                            ident_bf[:],
                        )
                        nc.any.tensor_copy(
                            woT_bf[:, b2, a * P:(a + 1) * P], pst[:]
                        )

                for t in range(2):
                    nc.any.tensor_copy(cuu[:, t, :], cps[t][:])
                nc.vector.tensor_copy(cuu_bf[:], cuu[:])

            with tc.tile_pool(name="sm", bufs=1) as sm:
                psA_ctx = tc.tile_pool(name="psA", bufs=1, space="PSUM")
                psA = psA_ctx.__enter__()
                a_k = sm.tile([P, 2, HD], BF16)
                m_k = sm.tile([P, 2, HD], F32)
                a_v = sm.tile([P, 2, HD], BF16)
                m_v = sm.tile([P, 2, HD], F32)
                for wT_t, a_t, mm_t in ((wkT_bf, a_k, m_k), (wvT_bf, a_v, m_v)):
                    for t in range(2):
                        aps = psA.tile([P, HD], F32, tag="aps", bufs=2, name="aps")
                        for tp in range(2):
                            nc.tensor.matmul(
                                aps[:],
                                cuu_bf[:, tp, t * P:(t + 1) * P],
                                wT_t[:, tp, :],
                                start=(tp == 0),
                                stop=(tp == 1),
                            )
                        nc.vector.tensor_copy(a_t[:, t, :], aps[:])
                        nc.vector.tensor_mul(mm_t[:, t, :], aps[:], wT_t[:, t, :])

                mk = sm.tile([1, HD], F32)
                mv = sm.tile([1, HD], F32)
                ekk = sm.tile([1, HD], F32)
                evv = sm.tile([1, HD], F32)
                tk = sm.tile([1, HD], F32)
                tv = sm.tile([1, HD], F32)
                vark = sm.tile([1, HD], F32)
                varv = sm.tile([1, HD], F32)
                for wT_t, m_t in ((wkT_bf, mk), (wvT_bf, mv)):
                    sps = psA.tile([1, HD], F32, tag="st", bufs=2, name="sps")
                    for tp in range(2):
                        nc.tensor.matmul(
                            sps[:],
                            cuu_bf[:, tp, C:C + 1],
                            wT_t[:, tp, :],
                            start=(tp == 0),
                            stop=(tp == 1),
                        )
                    nc.scalar.activation(m_t[:], sps[:], AF.Copy, scale=INV_N)
                for m_src, e_t in ((m_k, ekk), (m_v, evv)):
                    sps = psA.tile([1, HD], F32, tag="st", bufs=2, name="sps")
                    for tp in range(2):
                        nc.tensor.matmul(
                            sps[:],
                            ones_col_f[:],
                            m_src[:, tp, :],
                            start=(tp == 0),
                            stop=(tp == 1),
                        )
                    nc.scalar.activation(e_t[:], sps[:], AF.Copy, scale=INV_N)
                nc.vector.tensor_mul(tk[:], mk[:], mk[:])
                nc.vector.tensor_mul(tv[:], mv[:], mv[:])
                nc.vector.tensor_sub(vark[:], ekk[:], tk[:])
                nc.vector.tensor_sub(varv[:], evv[:], tv[:])

                eps_col = sm.tile([P, 4], F32)
                nc.vector.memset(eps_col[:], EPS)
                rk_col = sm.tile([P, 4], F32)
                rv_col = sm.tile([P, 4], F32)
                for var_row, r_col in ((vark, rk_col), (varv, rv_col)):
                    vc = psA.tile([P, 4], F32, tag="vc", bufs=2, name="vc")
                    for g in range(4):
                        nc.tensor.matmul(
                            vc[:, g:g + 1],
                            var_row[0:1, g * P:(g + 1) * P],
                            one1[:],
                            start=True,
                            stop=True,
                        )
                    nc.vector.tensor_add(r_col[:], vc[:], eps_col[:])
                    nc.scalar.activation(r_col[:], r_col[:], AF.Sqrt)
                    nc.vector.reciprocal(r_col[:], r_col[:])
                rk_row = sm.tile([1, HD], F32)
                rk_bc = sm.tile([P, HD], F32)
                rps = psA.tile([1, HD], F32, tag="st", bufs=2, name="rps")
                for g in range(4):
                    nc.tensor.matmul(
                        rps[0:1, g * P:(g + 1) * P],
                        rk_col[:, g:g + 1],
                        ident[:],
                        start=True,
                        stop=True,
                    )
                nc.scalar.mul(rk_row[:], rps[:], 1.0)
                bps = psA.tile([P, HD], F32, tag="aps", bufs=2, name="bps")
                nc.tensor.matmul(bps[:], ones_row[:], rk_row[:], start=True, stop=True)
                nc.any.tensor_copy(rk_bc[:], bps[:])
                psA_ctx.__exit__(None, None, None)

                with tc.tile_pool(name="psP", bufs=1, space="PSUM") as psP:
                    wps2 = [
                        psP.tile([P, OUT], F32, tag=f"weff{t}", name=f"wps{t}")
                        for t in range(2)
                    ]
                    for jp in range(4):
                        sl = slice(jp * P, (jp + 1) * P)
                        sd = psP.tile([P, P], F32, tag="sd", bufs=2, name="sd")
                        for tp in range(2):
                            nc.tensor.matmul(
                                sd[:],
                                wvT_bf[:, tp, sl],
                                a_k[:, tp, sl],
                                start=(tp == 0),
                                stop=(tp == 1),
                            )
                        outr = psP.tile([P, P], F32, tag="outr", bufs=2, name="outr")
                        nc.tensor.matmul(
                            outr[:], mv[:, sl], mk[:, sl], start=True, stop=True
                        )
                        kvp = sm.tile([P, P], F32, tag=f"kv{jp}", name=f"kv{jp}")
                        nc.vector.memset(kvp[:], 0.0)
                        for g in range(2):
                            gs = slice(g * 64, g * 64 + 64)
                            nc.scalar.mul(kvp[gs, gs], sd[gs, gs], INV_N)
                            nc.vector.tensor_sub(
                                kvp[gs, gs], kvp[gs, gs], outr[gs, gs]
                            )
                        nc.vector.tensor_mul(kvp[:], kvp[:], rk_bc[:, sl])
                        kvp_bf = sm.tile([P, P], BF16, tag=f"kvb{jp}", name=f"kvb{jp}")
                        nc.vector.tensor_scalar_mul(
                            kvp_bf[:], kvp[:], rv_col[:, jp:jp + 1]
                        )
                        bps2 = psP.tile([P, OUT], F32, tag="bps2", bufs=2, name="bps2")
                        nc.tensor.matmul(
                            bps2[:], kvp_bf[:], woT_bf[:, jp, :], start=True, stop=True
                        )
                        bsb = sm.tile([P, OUT], BF16, tag="bsb", name="bsb")
                        nc.any.tensor_copy(bsb[:], bps2[:])
                        for t in range(2):
                            nc.tensor.matmul(
                                wps2[t][:],
                                wq_bf[:, jp, t * P:(t + 1) * P],
                                bsb[:],
                                start=(jp == 0),
                                stop=(jp == 3),
                            )
                    for t in range(2):
                        nc.any.tensor_copy(weff[:, t, :], wps2[t][:])

            with (
                tc.tile_pool(name="opool", bufs=3) as opool,
                tc.tile_pool(name="pout", bufs=6, space="PSUM") as pout,
            ):
                for ch in range(ON_CHUNKS):
                    osb = opool.tile([P, OSUB, OUT], F32, tag="osb", name="osb")
                    for j in range(OSUB):
                        g = ch * OSUB + j
                        ops = pout.tile([P, OUT], F32, tag="ops", name="ops")
                        for t in range(2):
                            nc.tensor.matmul(
                                ops[:],
                                uT[:, t, g * P:(g + 1) * P],
                                weff[:, t, :],
                                start=(t == 0),
                                stop=(t == 1),
                            )
                        if j % 2 == 0:
                            nc.vector.tensor_copy(osb[:, j, :], ops[:])
                        else:
                            nc.scalar.mul(osb[:, j, :], ops[:], 1.0)
                    nc.sync.dma_start(
                        out_d[ch * OCH_ROWS:(ch + 1) * OCH_ROWS, :].rearrange(
                            "(p j) c -> p j c", p=P
                        ),
                        osb[:],
                    )

    nc.compile()
    return nc


_NC_CACHE = None


def _get_nc():
    global _NC_CACHE
    if _NC_CACHE is None:
        _NC_CACHE = build_nc()
    return _NC_CACHE


def make_in_maps(u_src, Wq, Wk, Wv, Wo):
    in_maps = []
    for c in range(8):
        b, half = c // 2, c % 2
        ub = u_src[b]
        mine = ub[half * N_HALF:(half + 1) * N_HALF]
        other = ub[(1 - half) * N_HALF:(2 - half) * N_HALF]
        u_perm = np.ascontiguousarray(np.concatenate([mine, other], axis=0))
        in_maps.append(
            {
                "u": u_perm,
                "wq": np.ascontiguousarray(Wq),
                "wk": np.ascontiguousarray(Wk),
                "wv": np.ascontiguousarray(Wv),
                "wo": np.ascontiguousarray(Wo),
            }
        )
    return in_maps


def assemble_output(results, bo):
    out = np.empty((4, N_FULL, OUT), dtype=np.float32)
    for c in range(8):
        b, half = c // 2, c % 2
        out[b, half * N_HALF:(half + 1) * N_HALF] = results[c]["out"]
    if np.any(bo):
        out += bo.reshape(1, 1, OUT)
    return out


def run(inputs, trace=False, tmpdir=None):
    u_src = np.asarray(inputs["u_src"], dtype=np.float32)
    Wq = np.asarray(inputs["Wq"], dtype=np.float32)
    Wk = np.asarray(inputs["Wk"], dtype=np.float32)
    Wv = np.asarray(inputs["Wv"], dtype=np.float32)
    Wo = np.asarray(inputs["Wo"], dtype=np.float32)
    bo = np.asarray(inputs["bo"], dtype=np.float32)
    nc = _get_nc()
    in_maps = make_in_maps(u_src, Wq, Wk, Wv, Wo)
    res = run_bass_kernel_spmd(
        nc, in_maps, core_ids=list(range(8)), trace=trace, tmpdir=tmpdir
    )
    return assemble_output(res.results, bo), res


def kernel(**inputs):
    out, _ = run(inputs, trace=False)
    return out
